# revision 4
# baseline (speedup 1.0000x reference)
import numpy as np

B = 8
SEQ = 4096
D = 1024
N_BASE = 10000.0
N_CORES = 8
SPC = SEQ // N_CORES   # 512 seq rows per core
JT = SPC // 128        # 4 rows per partition
BUFS = 5

# Slow-engine rebalance: DMA engine 15 serves partitions {92-95, 124-127}.
# Steal their slot-3 row in every chunk; route the stolen rows through an
# overflow tile on partitions [32:96] (4 partitions on every engine).
P_SLOW = [92, 93, 94, 95, 124, 125, 126, 127]
N_OVF = B * len(P_SLOW)       # 64 rows
A_ROWS = 128 * 3              # 384 rows per chunk (slots 0-2, all partitions)
B1_P0, B1_P1 = 0, 92          # slot-3 partitions, piece 1
B2_P0, B2_P1 = 96, 124        # slot-3 partitions, piece 2
CH_ROWS = A_ROWS + (B1_P1 - B1_P0) + (B2_P1 - B2_P0)  # 504
N_ROWS = B * CH_ROWS + N_OVF  # 4096

_CACHE = {}


def _compute_pe() -> np.ndarray:
    """Mirror of the reference _pos_encoding (default jax backend, f32)."""
    import jax
    import jax.numpy as jnp

    pos = jnp.arange(SEQ, dtype=jnp.float32)[:, None]
    i = jnp.arange(D // 2, dtype=jnp.float32)
    denom = jnp.power(jnp.float32(N_BASE), 2.0 * i / jnp.float32(D))
    ang = pos / denom
    pe = jnp.stack([jnp.sin(ang), jnp.cos(ang)], axis=-1).reshape(SEQ, D)
    return np.asarray(jax.device_get(pe), dtype=np.float32)


def _chunk_row_idx():
    """Row order (within a core's 512-row slice) for one chunk's pieces."""
    a = (4 * np.arange(128)[:, None] + np.arange(3)[None, :]).reshape(-1)
    b1 = 4 * np.arange(B1_P0, B1_P1) + 3
    b2 = 4 * np.arange(B2_P0, B2_P1) + 3
    return a, b1, b2


def _ovf_seq_idx():
    """(batch, within-core-seq-row) per overflow tile row j (j=8s+b)."""
    s = np.repeat(np.arange(len(P_SLOW)), B)
    b = np.tile(np.arange(B), len(P_SLOW))
    rows = 4 * np.asarray(P_SLOW)[s] + 3
    return b, rows


def _repack(x, c):
    xs = x[:, c * SPC : (c + 1) * SPC, :]  # [B, 512, D]
    a, b1, b2 = _chunk_row_idx()
    out = np.empty((N_ROWS, D), dtype=np.float32)
    for g in range(B):
        base = g * CH_ROWS
        out[base : base + A_ROWS] = xs[g][a]
        out[base + A_ROWS : base + A_ROWS + len(b1)] = xs[g][b1]
        out[base + A_ROWS + len(b1) : base + CH_ROWS] = xs[g][b2]
    ob, orow = _ovf_seq_idx()
    out[B * CH_ROWS :] = xs[ob, orow, :]
    return out


def _unpack(y, c, out):
    """Scatter core c's kernel output y [N_ROWS, D] into out [B, SEQ, D]."""
    dst = out[:, c * SPC : (c + 1) * SPC, :]
    a, b1, b2 = _chunk_row_idx()
    for g in range(B):
        base = g * CH_ROWS
        dst[g][a] = y[base : base + A_ROWS]
        dst[g][b1] = y[base + A_ROWS : base + A_ROWS + len(b1)]
        dst[g][b2] = y[base + A_ROWS + len(b1) : base + CH_ROWS]
    ob, orow = _ovf_seq_idx()
    dst[ob, orow, :] = y[B * CH_ROWS :]


def _pe_inputs(pe, c):
    pes = np.ascontiguousarray(pe[c * SPC : (c + 1) * SPC, :])
    ob, orow = _ovf_seq_idx()
    pe_ovf = np.ascontiguousarray(pes[orow])
    return pes, pe_ovf


def _build_program(bufs=BUFS):
    import concourse.bacc as bacc
    import concourse.mybir as mybir
    import concourse.tile as tile

    nc = bacc.Bacc("TRN2")
    f32 = mybir.dt.float32
    x_in = nc.declare_dram_parameter("x", [N_ROWS, D], f32, isOutput=False)
    pe_in = nc.declare_dram_parameter("pe", [SPC, D], f32, isOutput=False)
    po_in = nc.declare_dram_parameter("pe_ovf", [N_OVF, D], f32, isOutput=False)
    y_out = nc.declare_dram_parameter("y", [N_ROWS, D], f32, isOutput=True)

    nb1 = B1_P1 - B1_P0
    nb2 = B2_P1 - B2_P0

    with tile.TileContext(nc) as tc:
        with (
            tc.tile_pool(name="pe_pool", bufs=1) as pe_pool,
            tc.tile_pool(name="x_pool", bufs=bufs) as x_pool,
            tc.tile_pool(name="ovf_pool", bufs=1) as ovf_pool,
        ):
            pe_t = pe_pool.tile([128, JT, D], f32)
            nc.sync.dma_start(
                out=pe_t[:], in_=pe_in.rearrange("(p u) d -> p u d", u=JT)
            )
            po_t = ovf_pool.tile([128, 1, D], f32, tag="po")
            nc.sync.dma_start(
                out=po_t[32:96], in_=po_in.rearrange("(p u) d -> p u d", u=1)
            )
            ov_t = ovf_pool.tile([128, 1, D], f32, tag="ov")
            nc.sync.dma_start(
                out=ov_t[32:96],
                in_=x_in[B * CH_ROWS :, :].rearrange("(p u) d -> p u d", u=1),
            )
            nc.vector.tensor_add(ov_t[32:64], ov_t[32:64], po_t[32:64])
            nc.vector.tensor_add(ov_t[64:96], ov_t[64:96], po_t[64:96])
            nc.scalar.dma_start(
                out=y_out[B * CH_ROWS :, :].rearrange("(p u) d -> p u d", u=1),
                in_=ov_t[32:96],
            )
            for g in range(B):
                base = g * CH_ROWS
                xt = x_pool.tile([128, JT, D], f32)
                nc.sync.dma_start(
                    out=xt[:, 0:3, :],
                    in_=x_in[base : base + A_ROWS, :].rearrange(
                        "(p k) d -> p k d", k=3
                    ),
                )
                nc.sync.dma_start(
                    out=xt[B1_P0:B1_P1, 3:4, :],
                    in_=x_in[base + A_ROWS : base + A_ROWS + nb1, :].rearrange(
                        "(p k) d -> p k d", k=1
                    ),
                )
                nc.sync.dma_start(
                    out=xt[B2_P0:B2_P1, 3:4, :],
                    in_=x_in[
                        base + A_ROWS + nb1 : base + CH_ROWS, :
                    ].rearrange("(p k) d -> p k d", k=1),
                )
                nc.vector.tensor_add(
                    xt[:, 0:3, :], xt[:, 0:3, :], pe_t[:, 0:3, :]
                )
                nc.vector.tensor_add(
                    xt[B1_P0:B1_P1, 3:4, :],
                    xt[B1_P0:B1_P1, 3:4, :],
                    pe_t[B1_P0:B1_P1, 3:4, :],
                )
                nc.vector.tensor_add(
                    xt[B2_P0:B2_P1, 3:4, :],
                    xt[B2_P0:B2_P1, 3:4, :],
                    pe_t[B2_P0:B2_P1, 3:4, :],
                )
                nc.scalar.dma_start(
                    out=y_out[base : base + A_ROWS, :].rearrange(
                        "(p k) d -> p k d", k=3
                    ),
                    in_=xt[:, 0:3, :],
                )
                nc.scalar.dma_start(
                    out=y_out[base + A_ROWS : base + A_ROWS + nb1, :].rearrange(
                        "(p k) d -> p k d", k=1
                    ),
                    in_=xt[B1_P0:B1_P1, 3:4, :],
                )
                nc.scalar.dma_start(
                    out=y_out[
                        base + A_ROWS + nb1 : base + CH_ROWS, :
                    ].rearrange("(p k) d -> p k d", k=1),
                    in_=xt[B2_P0:B2_P1, 3:4, :],
                )
    if not nc.is_finalized():
        nc.finalize()
    return nc


def _get_state():
    if "nc" not in _CACHE:
        _CACHE["nc"] = _build_program()
    if "pe" not in _CACHE:
        _CACHE["pe"] = _compute_pe()
    return _CACHE["nc"], _CACHE["pe"]


def _make_in_maps(x):
    nc, pe = _get_state()
    in_maps = []
    for c in range(N_CORES):
        pes, pe_ovf = _pe_inputs(pe, c)
        in_maps.append({"x": _repack(x, c), "pe": pes, "pe_ovf": pe_ovf})
    return nc, in_maps


def kernel(x, seq_len=None, **_):
    from concourse.bass_utils import run_bass_kernel_spmd

    x = np.asarray(x, dtype=np.float32)
    assert x.shape == (B, SEQ, D)
    if seq_len is not None:
        assert int(np.asarray(seq_len)) == SEQ

    nc, in_maps = _make_in_maps(x)
    res = run_bass_kernel_spmd(nc, in_maps, list(range(N_CORES))).results

    out = np.empty((B, SEQ, D), dtype=np.float32)
    for c in range(N_CORES):
        _unpack(res[c]["y"], c, out)
    return out


# revision 6
# speedup vs baseline: 1.3204x; 1.3204x over previous
import numpy as np

B = 8
SEQ = 4096
D = 1024
N_BASE = 10000.0
N_CORES = 8
SPC = SEQ // N_CORES   # 512 seq rows per core
JT = SPC // 128        # 4 rows per partition
BUFS = 5

# HWDGE splits a transfer's descriptors equally over the largest divisor of
# the descriptor count <= 16, as sequential blocks starting at engine 0.
# A 120-descriptor transfer therefore uses engines 0-14 and leaves engine 15
# (a frequent cross-core-contention straggler) idle. Steal the slot-3 row of
# partitions [120:128] in every chunk; the stolen rows ride a [64,1,D]
# overflow tile (64 descs -> all 16 engines, 4 each).
P_STEAL0 = 120          # partitions [P_STEAL0:128] donate slot 3
N_OVF = B * (128 - P_STEAL0)          # 64 rows
A_ROWS = 128 * 3                      # slots 0-2, all partitions
B_ROWS = P_STEAL0                     # slot-3 rows on partitions [0:120]
CH_ROWS = A_ROWS + B_ROWS             # 504
N_ROWS = B * CH_ROWS + N_OVF          # 4096

_CACHE = {}


def _compute_pe() -> np.ndarray:
    """Mirror of the reference _pos_encoding (default jax backend, f32)."""
    import jax
    import jax.numpy as jnp

    pos = jnp.arange(SEQ, dtype=jnp.float32)[:, None]
    i = jnp.arange(D // 2, dtype=jnp.float32)
    denom = jnp.power(jnp.float32(N_BASE), 2.0 * i / jnp.float32(D))
    ang = pos / denom
    pe = jnp.stack([jnp.sin(ang), jnp.cos(ang)], axis=-1).reshape(SEQ, D)
    return np.asarray(jax.device_get(pe), dtype=np.float32)


def _chunk_row_idx():
    """Row order (within a core's 512-row slice) for one chunk's pieces."""
    a = (4 * np.arange(128)[:, None] + np.arange(3)[None, :]).reshape(-1)
    b = 4 * np.arange(0, P_STEAL0) + 3
    return a, b


def _ovf_seq_idx():
    """(batch, within-core-seq-row) per overflow tile row j (j=8s+b)."""
    ns = 128 - P_STEAL0
    s = np.repeat(np.arange(ns), B)
    b = np.tile(np.arange(B), ns)
    rows = 4 * (P_STEAL0 + s) + 3
    return b, rows


def _repack(x, c):
    xs = x[:, c * SPC : (c + 1) * SPC, :]  # [B, 512, D]
    a, bidx = _chunk_row_idx()
    out = np.empty((N_ROWS, D), dtype=np.float32)
    for g in range(B):
        base = g * CH_ROWS
        out[base : base + A_ROWS] = xs[g][a]
        out[base + A_ROWS : base + CH_ROWS] = xs[g][bidx]
    ob, orow = _ovf_seq_idx()
    out[B * CH_ROWS :] = xs[ob, orow, :]
    return out


def _unpack(y, c, out):
    """Scatter core c's kernel output y [N_ROWS, D] into out [B, SEQ, D]."""
    dst = out[:, c * SPC : (c + 1) * SPC, :]
    a, bidx = _chunk_row_idx()
    for g in range(B):
        base = g * CH_ROWS
        dst[g][a] = y[base : base + A_ROWS]
        dst[g][bidx] = y[base + A_ROWS : base + CH_ROWS]
    ob, orow = _ovf_seq_idx()
    dst[ob, orow, :] = y[B * CH_ROWS :]


def _pe_inputs(pe, c):
    pes = np.ascontiguousarray(pe[c * SPC : (c + 1) * SPC, :])
    ob, orow = _ovf_seq_idx()
    pe_ovf = np.ascontiguousarray(pes[orow])
    return pes, pe_ovf


def _build_program(bufs=BUFS):
    import concourse.bacc as bacc
    import concourse.mybir as mybir
    import concourse.tile as tile

    nc = bacc.Bacc("TRN2")
    f32 = mybir.dt.float32
    x_in = nc.declare_dram_parameter("x", [N_ROWS, D], f32, isOutput=False)
    pe_in = nc.declare_dram_parameter("pe", [SPC, D], f32, isOutput=False)
    po_in = nc.declare_dram_parameter("pe_ovf", [N_OVF, D], f32, isOutput=False)
    y_out = nc.declare_dram_parameter("y", [N_ROWS, D], f32, isOutput=True)

    with tile.TileContext(nc) as tc:
        with (
            tc.tile_pool(name="pe_pool", bufs=1) as pe_pool,
            tc.tile_pool(name="x_pool", bufs=bufs) as x_pool,
            tc.tile_pool(name="ovf_pool", bufs=1) as ovf_pool,
        ):
            pe_t = pe_pool.tile([128, JT, D], f32)
            nc.sync.dma_start(
                out=pe_t[:], in_=pe_in.rearrange("(p u) d -> p u d", u=JT)
            )
            po_t = ovf_pool.tile([N_OVF, 1, D], f32, tag="po")
            nc.sync.dma_start(
                out=po_t[:], in_=po_in.rearrange("(p u) d -> p u d", u=1)
            )
            ov_t = ovf_pool.tile([N_OVF, 1, D], f32, tag="ov")
            nc.sync.dma_start(
                out=ov_t[:],
                in_=x_in[B * CH_ROWS :, :].rearrange("(p u) d -> p u d", u=1),
            )
            nc.vector.tensor_add(ov_t[:], ov_t[:], po_t[:])
            nc.scalar.dma_start(
                out=y_out[B * CH_ROWS :, :].rearrange("(p u) d -> p u d", u=1),
                in_=ov_t[:],
            )
            for g in range(B):
                base = g * CH_ROWS
                xt = x_pool.tile([128, JT, D], f32)
                nc.sync.dma_start(
                    out=xt[:, 0:3, :],
                    in_=x_in[base : base + A_ROWS, :].rearrange(
                        "(p k) d -> p k d", k=3
                    ),
                )
                nc.sync.dma_start(
                    out=xt[0:P_STEAL0, 3:4, :],
                    in_=x_in[base + A_ROWS : base + CH_ROWS, :].rearrange(
                        "(p k) d -> p k d", k=1
                    ),
                )
                nc.vector.tensor_add(
                    xt[:, 0:3, :], xt[:, 0:3, :], pe_t[:, 0:3, :]
                )
                nc.vector.tensor_add(
                    xt[0:P_STEAL0, 3:4, :],
                    xt[0:P_STEAL0, 3:4, :],
                    pe_t[0:P_STEAL0, 3:4, :],
                )
                nc.scalar.dma_start(
                    out=y_out[base : base + A_ROWS, :].rearrange(
                        "(p k) d -> p k d", k=3
                    ),
                    in_=xt[:, 0:3, :],
                )
                nc.scalar.dma_start(
                    out=y_out[base + A_ROWS : base + CH_ROWS, :].rearrange(
                        "(p k) d -> p k d", k=1
                    ),
                    in_=xt[0:P_STEAL0, 3:4, :],
                )
    if not nc.is_finalized():
        nc.finalize()
    return nc


def _get_state():
    if "nc" not in _CACHE:
        _CACHE["nc"] = _build_program()
    if "pe" not in _CACHE:
        _CACHE["pe"] = _compute_pe()
    return _CACHE["nc"], _CACHE["pe"]


def _make_in_maps(x):
    nc, pe = _get_state()
    in_maps = []
    for c in range(N_CORES):
        pes, pe_ovf = _pe_inputs(pe, c)
        in_maps.append({"x": _repack(x, c), "pe": pes, "pe_ovf": pe_ovf})
    return nc, in_maps


def kernel(x, seq_len=None, **_):
    from concourse.bass_utils import run_bass_kernel_spmd

    x = np.asarray(x, dtype=np.float32)
    assert x.shape == (B, SEQ, D)
    if seq_len is not None:
        assert int(np.asarray(seq_len)) == SEQ

    nc, in_maps = _make_in_maps(x)
    res = run_bass_kernel_spmd(nc, in_maps, list(range(N_CORES))).results

    out = np.empty((B, SEQ, D), dtype=np.float32)
    for c in range(N_CORES):
        _unpack(res[c]["y"], c, out)
    return out
